# revision 1
# baseline (speedup 1.0000x reference)
"""GraphSAGE (5-layer, mean aggr) on 8 Trainium2 NeuronCores via Bass/Tile.

Strategy (matches the sharding hint):
  - Nodes are sharded contiguously across the 8 cores (12500 nodes each);
    each core owns all edges whose *destination* falls in its shard.
  - Per layer, each core computes y = h @ Wl for its own nodes; an AllGather
    replicates y so that every core can gather arbitrary source-neighbor
    rows (the "halo exchange" -- here the halo is effectively global since
    the graph is random).
  - Aggregation: edges are grouped by 128-node destination blocks; for each
    128-edge sub-tile we gather the 128 source rows of y with one slice of a
    block-level indirect DMA, build a one-hot selection matrix S[e, d] =
    (dst_local[e] == d) on the VectorEngine, and accumulate S^T @ G into
    PSUM on the TensorEngine.  The self term (h @ Wr + bl, pre-scaled by
    max(deg,1)) is added with one vector add, then
    h' = relu(inv_deg * psum) on the ScalarEngine.
  - Features travel as bf16 (halves the dominant random-gather traffic);
    all matmul accumulation is fp32 in PSUM.
"""

import sys
import os
import numpy as np

for _p in ("/opt/trn_rl_repo",):
    if _p not in sys.path and os.path.isdir(_p):
        sys.path.insert(0, _p)

import ml_dtypes  # noqa: E402

BF16 = ml_dtypes.bfloat16

# ---------------------------------------------------------------- constants
N_NODES = 100000
N_EDGES = 1600000
HID = 64
N_LAYERS = 5
N_CORES = 8
P = 128


# ---------------------------------------------------------------- host prep
def prep_host(x, edge_index, Wl, bl, Wr, Wfc, bfc, n_nodes, n_cores):
    """Graph partitioning + per-core table construction (pure numpy)."""
    npc = n_nodes // n_cores          # nodes per core
    nblk = (npc + P - 1) // P         # dst blocks per core
    npcp = nblk * P                   # padded nodes per core

    src = edge_index[0].astype(np.int64)
    dst = edge_index[1].astype(np.int64)
    deg = np.bincount(dst, minlength=n_nodes).astype(np.float32)
    degc = np.maximum(deg, 1.0)
    inv = (1.0 / degc).astype(np.float32)

    order = np.argsort(dst, kind="stable")
    ds = dst[order]
    ss = src[order]
    # remap src node id -> padded row id in the all-gathered y table
    ssr = ((ss // npc) * npcp + (ss % npc)).astype(np.int32)

    lcl = ds % npc
    core_of = ds // npc
    gblk = (core_of * nblk + lcl // P).astype(np.int64)
    dloc = (lcl % P).astype(np.float32)

    cnt = np.bincount(gblk, minlength=n_cores * nblk)
    kb = np.ceil(cnt.reshape(n_cores, nblk) / P).astype(np.int64).max(axis=0)
    kb = np.maximum(kb, 1)            # per-block sub-tile count (max over cores)
    off = np.zeros(nblk, np.int64)
    off[1:] = np.cumsum(kb)[:-1]
    kt = int(kb.sum())

    idx = np.zeros((n_cores, P, kt), np.int32)
    dstl = np.full((n_cores, P, kt), 200.0, np.float32)  # 200 => pad (no match)
    starts = np.zeros(n_cores * nblk + 1, np.int64)
    starts[1:] = np.cumsum(cnt)
    for c in range(n_cores):
        for b in range(nblk):
            g = c * nblk + b
            s0, s1 = starts[g], starts[g + 1]
            e = s1 - s0
            if e == 0:
                continue
            j = np.arange(e)
            pp = j % P
            kk = j // P + off[b]
            idx[c, pp, kk] = ssr[s0:s1]
            dstl[c, pp, kk] = dloc[s0:s1]

    # per-core per-block node scales, block-major [P, nblk]
    scl = np.ones((n_cores, P, nblk), np.float32)
    dgc = np.ones((n_cores, P, nblk), np.float32)
    xpad = np.zeros((n_cores, npcp, HID), np.float32)
    for c in range(n_cores):
        nid = c * npc + np.arange(npc)
        pp = np.arange(npc) % P
        bb = np.arange(npc) // P
        scl[c, pp, bb] = inv[nid]
        dgc[c, pp, bb] = degc[nid]
        xpad[c, :npc] = x[c * npc:(c + 1) * npc]

    # weights, SBUF-layout
    nl = Wl.shape[0]
    wl_h = np.zeros((HID, nl * HID), np.float32)   # [f, l*64+f'] = Wl[l,f,f']
    wr_h = np.zeros((HID, nl * HID), np.float32)
    bl_h = np.zeros((P, nl * HID), np.float32)     # replicated across partitions
    for l in range(nl):
        wl_h[:, l * HID:(l + 1) * HID] = Wl[l]
        wr_h[:, l * HID:(l + 1) * HID] = Wr[l]
        bl_h[:, l * HID:(l + 1) * HID] = bl[l][None, :]
    wfct_h = Wfc.reshape(5, HID).T.astype(np.float32)    # [64, 5]
    iota_h = np.broadcast_to(np.arange(P, dtype=np.float32), (P, P)).copy()
    ident_h = np.eye(P, dtype=np.float32)

    ng = npc // 5                      # graphs per core
    gb = (ng + P - 1) // P             # head groups per core

    in_maps = []
    for c in range(n_cores):
        in_maps.append({
            "x_in": xpad[c].astype(np.float32),
            "idx_in": idx[c],
            "dstl_in": dstl[c].astype(np.float32),
            "scl_in": scl[c],
            "dgc_in": dgc[c],
            "iota_in": iota_h.astype(BF16),
            "ident_in": ident_h.astype(BF16),
            "wl_in": wl_h.astype(BF16),
            "wr_in": wr_h.astype(BF16),
            "bl_in": bl_h.astype(BF16),
            "wfct_in": wfct_h.astype(BF16),
        })
    params = dict(kb=[int(v) for v in kb], off=[int(v) for v in off], kt=kt,
                  nblk=nblk, npcp=npcp, npc=npc, ng=ng, gb=gb,
                  bfc=float(np.asarray(bfc).reshape(-1)[0]))
    return in_maps, params


# ---------------------------------------------------------------- program
def build_program(nc, params, n_cores, reps=1, variant='full'):
    import concourse.bass as bass
    import concourse.tile as tile
    from concourse import mybir
    from contextlib import ExitStack

    f32 = mybir.dt.float32
    bf16 = mybir.dt.bfloat16
    i32 = mybir.dt.int32
    AF = mybir.ActivationFunctionType
    OP = mybir.AluOpType

    kb, off, kt = params["kb"], params["off"], params["kt"]
    nblk, npcp, ng, gb = params["nblk"], params["npcp"], params["ng"], params["gb"]
    bfc = params["bfc"]
    nl = N_LAYERS

    x_in = nc.dram_tensor("x_in", [npcp, HID], f32, kind="ExternalInput")
    idx_in = nc.dram_tensor("idx_in", [P, kt], i32, kind="ExternalInput")
    dstl_in = nc.dram_tensor("dstl_in", [P, kt], f32, kind="ExternalInput")
    scl_in = nc.dram_tensor("scl_in", [P, nblk], f32, kind="ExternalInput")
    dgc_in = nc.dram_tensor("dgc_in", [P, nblk], f32, kind="ExternalInput")
    iota_in = nc.dram_tensor("iota_in", [P, P], bf16, kind="ExternalInput")
    ident_in = nc.dram_tensor("ident_in", [P, P], bf16, kind="ExternalInput")
    wl_in = nc.dram_tensor("wl_in", [HID, nl * HID], bf16, kind="ExternalInput")
    wr_in = nc.dram_tensor("wr_in", [HID, nl * HID], bf16, kind="ExternalInput")
    bl_in = nc.dram_tensor("bl_in", [P, nl * HID], bf16, kind="ExternalInput")
    wfct_in = nc.dram_tensor("wfct_in", [HID, 5], bf16, kind="ExternalInput")
    out_t = nc.dram_tensor("out", [P, gb], f32, kind="ExternalOutput")

    groups = [list(range(n_cores))]

    with tile.TileContext(nc) as tc:
        _frees = []
        with ExitStack() as ctx:
            # ---- persistent SBUF (freed LIFO after the pools close)
            idx_sb, _f = tc.tile([P, kt], i32, name="idx_sb"); _frees.append(_f)
            dstl_sb, _f = tc.tile([P, kt], f32, name="dstl_sb"); _frees.append(_f)
            scl_sb, _f = tc.tile([P, nblk], f32, name="scl_sb"); _frees.append(_f)
            dgc_sb, _f = tc.tile([P, nblk], f32, name="dgc_sb"); _frees.append(_f)
            iota_sb, _f = tc.tile([P, P], bf16, name="iota_sb"); _frees.append(_f)
            ident_sb, _f = tc.tile([P, P], bf16, name="ident_sb"); _frees.append(_f)
            wl_sb, _f = tc.tile([HID, nl * HID], bf16, name="wl_sb"); _frees.append(_f)
            wr_sb, _f = tc.tile([HID, nl * HID], bf16, name="wr_sb"); _frees.append(_f)
            bl_sb, _f = tc.tile([P, nl * HID], bf16, name="bl_sb"); _frees.append(_f)
            wfct_sb, _f = tc.tile([HID, 5], bf16, name="wfct_sb"); _frees.append(_f)
            z0_sb, _f = tc.tile([P, nblk * HID], bf16, name="z0_sb"); _frees.append(_f)
            z1_sb, _f = tc.tile([P, nblk * HID], bf16, name="z1_sb"); _frees.append(_f)
            h5t_sb, _f = tc.tile([HID, nblk * P], bf16, name="h5t_sb"); _frees.append(_f)

            nc.sync.dma_start(idx_sb[:], idx_in[:])
            nc.sync.dma_start(dstl_sb[:], dstl_in[:])
            nc.sync.dma_start(scl_sb[:], scl_in[:])
            nc.sync.dma_start(dgc_sb[:], dgc_in[:])
            nc.sync.dma_start(iota_sb[:], iota_in[:])
            nc.sync.dma_start(ident_sb[:], ident_in[:])
            nc.sync.dma_start(wl_sb[:], wl_in[:])
            nc.sync.dma_start(wr_sb[:], wr_in[:])
            nc.sync.dma_start(bl_sb[:], bl_in[:])
            nc.sync.dma_start(wfct_sb[:], wfct_in[:])

            # ---- pools
            dram = ctx.enter_context(tc.tile_pool(name="dram", bufs=1, space="DRAM"))
            sb = ctx.enter_context(tc.tile_pool(name="sb", bufs=3))
            gp = ctx.enter_context(tc.tile_pool(name="gp", bufs=3))
            ps = ctx.enter_context(tc.tile_pool(name="ps", bufs=2, space="PSUM"))

            def transpose_h(hb_ap, dst_slice=None):
                t_ps = ps.tile([HID, P], bf16, tag="tps", name="t_ps")
                nc.tensor.transpose(t_ps[:], hb_ap, ident_sb[:])
                if dst_slice is None:
                    hbt = sb.tile([HID, P], bf16, tag="hbt", name="hbt", bufs=4)
                    nc.vector.tensor_copy(hbt[:], t_ps[:])
                    return hbt
                nc.vector.tensor_copy(dst_slice, t_ps[:])
                return None

            def produce_yz(hbt_ap, lw, b, y_own, zout):
                y_ps = ps.tile([P, HID], f32, tag="yzps", name="y_ps")
                nc.tensor.matmul(y_ps[:], lhsT=hbt_ap,
                                 rhs=wl_sb[:, lw * HID:(lw + 1) * HID],
                                 start=True, stop=True)
                yb = sb.tile([P, HID], bf16, tag="yb", name="yb", bufs=4)
                nc.scalar.copy(yb[:], y_ps[:])
                nc.sync.dma_start(y_own[b * P:(b + 1) * P, :], yb[:])
                z_ps = ps.tile([P, HID], f32, tag="yzps", name="z_ps")
                nc.tensor.matmul(z_ps[:], lhsT=hbt_ap,
                                 rhs=wr_sb[:, lw * HID:(lw + 1) * HID],
                                 start=True, stop=True)
                nc.vector.tensor_tensor(
                    out=z_ps[:], in0=z_ps[:],
                    in1=bl_sb[:, lw * HID:(lw + 1) * HID], op=OP.add)
                nc.scalar.activation(zout[:, b * HID:(b + 1) * HID], z_ps[:],
                                     AF.Copy, scale=dgc_sb[:, b:b + 1])

            # ---- bootstrap: h0 = x -> y0, z0
            for _rep in range(reps):
              y_own = dram.tile([npcp, HID], bf16, tag="yown", name="y_own_b")
              for b in range(nblk):
                  xb = sb.tile([P, HID], bf16, tag="xb", name="xb")
                  nc.gpsimd.dma_start(xb[:], x_in[b * P:(b + 1) * P, :])
                  hbt = transpose_h(xb[:])
                  produce_yz(hbt[:], 0, b, y_own, z0_sb)
              y_full = dram.tile([n_cores * npcp, HID], bf16, tag="yfull",
                                 name="y_full_b")
              if variant == 'no_coll':
                  nc.sync.dma_start(y_full[:npcp, :], y_own[:])
              else:
                  nc.gpsimd.collective_compute(
                      "AllGather", OP.bypass, replica_groups=groups,
                      ins=[y_own.opt()], outs=[y_full.opt()])

              zin, zout = z0_sb, z1_sb
              for l in range(nl):
                  last = l == nl - 1
                  if not last:
                      y_own = dram.tile([npcp, HID], bf16, tag="yown",
                                        name=f"y_own_{l}")
                  for b in range(nblk):
                      k = kb[b]
                      o = off[b]
                      g_ts = []
                      for kk in range(k):
                          g_t = gp.tile([P, HID], bf16, tag="g", name="g_t")
                          g_ts.append(g_t)
                          if variant == 'direct_gather':
                              r0 = ((b * 23 + kk * 7) % 700) * P
                              nc.sync.dma_start(g_t[:], y_full[r0:r0 + P, :])
                          else:
                              nc.gpsimd.indirect_dma_start(
                                  out=g_t[:],
                                  out_offset=None, in_=y_full[:],
                                  in_offset=bass.IndirectOffsetOnAxis(
                                      ap=idx_sb[:, o + kk:o + kk + 1], axis=0))
                      a_ps = ps.tile([P, HID], f32, tag="aps", name="a_ps", bufs=3)
                      nsub = 1 if variant == 'gathers_only' else k
                      for kk in range(nsub):
                          s_t = sb.tile([P, P], bf16, tag="s", name="s_t", bufs=24)
                          nc.vector.tensor_scalar(
                              s_t[:], iota_sb[:], dstl_sb[:, o + kk:o + kk + 1],
                              None, op0=OP.is_equal)
                          nc.tensor.matmul(a_ps[:], lhsT=s_t[:],
                                           rhs=g_ts[kk][:],
                                           start=(kk == 0), stop=(kk == nsub - 1))
                      nc.vector.tensor_tensor(
                          out=a_ps[:], in0=a_ps[:],
                          in1=zin[:, b * HID:(b + 1) * HID], op=OP.add)
                      hb = sb.tile([P, HID], bf16, tag="hb", name="hb", bufs=4)
                      nc.scalar.activation(hb[:], a_ps[:], AF.Relu,
                                           scale=scl_sb[:, b:b + 1])
                      if last:
                          transpose_h(hb[:], dst_slice=h5t_sb[:, b * P:(b + 1) * P])
                      else:
                          hbt = transpose_h(hb[:])
                          produce_yz(hbt[:], l + 1, b, y_own, zout)
                  if not last:
                      y_full = dram.tile([n_cores * npcp, HID], bf16, tag="yfull",
                                         name=f"y_full_{l}")
                      nc.gpsimd.collective_compute(
                          "AllGather", OP.bypass, replica_groups=groups,
                          ins=[y_own.opt()], outs=[y_full.opt()])
                      zin, zout = zout, zin

              # ---- head: out[g] = sigmoid(sum_j h5[5g+j] . wfc_j + bfc)
              hd_ps = ps.tile([P, gb], f32, tag="aps", name="hd_ps", bufs=3)
              for t in range(gb):
                  gcnt = min(P, ng - t * P)
                  for j in range(5):
                      c0 = 5 * t * P + j
                      lhsT = h5t_sb[:, c0:c0 + 5 * gcnt - 4:5]
                      nc.tensor.matmul(hd_ps[:gcnt, t:t + 1], lhsT=lhsT,
                                       rhs=wfct_sb[:, j:j + 1],
                                       start=(j == 0), stop=(j == 4))
              out_sb = sb.tile([P, gb], f32, tag="outsb", name="out_sb")
              bfc_sb = sb.tile([P, 1], f32, tag="bfc", name="bfc_sb")
              nc.vector.memset(bfc_sb[:], bfc)
              nc.scalar.activation(out_sb[:], hd_ps[:], AF.Sigmoid, bias=bfc_sb[:])
              nc.sync.dma_start(out_t[:], out_sb[:])

        for _f in reversed(_frees):
            _f()

    return out_t


def make_nc(params, n_cores, enable_asserts=False, reps=1, variant='full'):
    import concourse.bacc as bacc
    nc = bacc.Bacc("TRN2", target_bir_lowering=False, debug=False,
                   enable_asserts=enable_asserts, num_devices=n_cores)
    build_program(nc, params, n_cores, reps=reps, variant=variant)
    nc.compile()
    return nc


def assemble_output(results, params, n_cores):
    """results: list (per core) of dicts with 'out' [P, gb] f32."""
    ng, gb = params["ng"], params["gb"]
    out = np.zeros((n_cores * ng, 1), np.float32)
    for c in range(n_cores):
        o = np.asarray(results[c]["out"])          # [P, gb]
        flat = o.T.reshape(-1)[:ng]                # graph g = t*P + p
        out[c * ng:(c + 1) * ng, 0] = flat
    return out


# ---------------------------------------------------------------- entry
def kernel(x, edge_index, Wl, bl, Wr, Wfc, bfc):
    from concourse.bass_utils import run_bass_kernel_spmd

    x = np.asarray(x, dtype=np.float32)
    edge_index = np.asarray(edge_index, dtype=np.int32)
    in_maps, params = prep_host(x, edge_index, np.asarray(Wl), np.asarray(bl),
                                np.asarray(Wr), np.asarray(Wfc),
                                np.asarray(bfc), x.shape[0], N_CORES)
    nc = make_nc(params, N_CORES)
    res = run_bass_kernel_spmd(nc, in_maps, core_ids=list(range(N_CORES)))
    return assemble_output(res.results, params, N_CORES)



# revision 32
# speedup vs baseline: 587.3187x; 587.3187x over previous
"""GraphSAGE (5-layer, mean aggr) on 8 Trainium2 NeuronCores via Bass/Tile.

Strategy (matches the sharding hint):
  - Nodes are sharded contiguously across the 8 cores (12500 nodes each);
    each core owns all edges whose *destination* falls in its shard.
  - Per layer, each core computes y = h @ Wl for its own nodes; an AllGather
    (split into two halves so the first half overlaps the tail of the
    previous layer's compute) replicates y so that every core can gather
    arbitrary source-neighbor rows.  The collective trigger blocks the
    issuing engine until ncfw signals completion; explicit dep edges also
    gate the gathers on it.
  - Aggregation: neighbor rows are fetched with the dma_gather custom
    instruction (InstDMAGatherAnt): thousands of int16 indices per
    instruction at ~0.34ns/descriptor of Pool-engine time, vs ~1us fixed
    per generic indirect DMA (which hardware limits to 128 rows per call).
    dma_gather requires 256B elements, so table rows are padded to 128
    bf16 values (only the first 64 are real), and int16 indexing splits
    the table into four 25088-row quarter views.  Slot columns are laid
    out chunk-major (7-dst-block chunk -> quarter -> block) so each
    (chunk, quarter) is one dma_gather over a contiguous index range.
  - Per 128-edge sub-tile a one-hot matrix S[e, d] = (dst_local[e] == d)
    is built on the VectorEngine and S^T @ G accumulated into PSUM on the
    TensorEngine; the self term h @ Wr + bl is accumulated into the same
    PSUM tile by two more matmuls.  The resident transposed feature buffer
    holds hd = deg * h (so hd' = relu(PSUM) without any scale), inv-deg is
    applied in f32 on the y-producing copy / final-layer relu, and the
    one-hot stays an exact 0/1 matrix.
"""

import sys
import os
import numpy as np

for _p in ("/opt/trn_rl_repo",):
    if _p not in sys.path and os.path.isdir(_p):
        sys.path.insert(0, _p)

import ml_dtypes  # noqa: E402

BF16 = ml_dtypes.bfloat16

# ---------------------------------------------------------------- constants
N_NODES = 100000
N_EDGES = 1600000
HID = 64
HID2 = 128       # padded table row width (bf16) -> 256B dma_gather elements
N_LAYERS = 5
N_CORES = 8
P = 128
GB = 5           # dst blocks per gather chunk
NQ = 4           # table quarters (int16 index range)


# ---------------------------------------------------------------- host prep
def prep_host(x, edge_index, Wl, bl, Wr, Wfc, bfc, n_nodes, n_cores):
    """Graph partitioning + per-core table construction (pure numpy)."""
    npc = n_nodes // n_cores          # nodes per core
    nblk = (npc + P - 1) // P         # dst blocks per core
    npcp = nblk * P                   # padded nodes per core
    hnp = npcp // 2                   # half (for the split AllGather)
    nfull = n_cores * npcp
    qrows = nfull // NQ               # rows per table quarter

    src = edge_index[0].astype(np.int64)
    dst = edge_index[1].astype(np.int64)
    deg = np.bincount(dst, minlength=n_nodes).astype(np.float32)
    degc = np.maximum(deg, 1.0)
    inv = (1.0 / degc).astype(np.float32)

    # table row of a source node (split-half AllGather layout)
    s_core = src // npc
    s_loc = src % npc
    ssr_all = np.where(s_loc < hnp,
                       s_core * hnp + s_loc,
                       n_cores * hnp + s_core * hnp + (s_loc - hnp))

    lcl = dst % npc
    core_of = dst // npc
    blk = lcl // P
    dloc = (lcl % P).astype(np.float32)
    qtr = ssr_all // qrows
    chunk = blk // GB
    nch = (nblk + GB - 1) // GB

    # sort edges by (core, chunk, quarter, block)
    order = np.lexsort((blk, qtr, chunk, core_of))
    co = core_of[order]
    bo = blk[order]
    qo = qtr[order]
    so = (ssr_all[order] % qrows).astype(np.int32)
    do = dloc[order]

    key = ((co * nblk + bo) * NQ + qo)
    cnt = np.bincount(key, minlength=n_cores * nblk * NQ)
    cntr = cnt.reshape(n_cores, nblk, NQ)
    kbq = np.ceil(cntr / P).astype(np.int64).max(axis=0)   # [nblk, NQ]
    kbq[:, 0] = np.maximum(kbq[:, 0], 1)                   # >=1 subtile/block

    # chunk-major column layout: chunk -> quarter -> block -> subtiles
    off2 = np.zeros((nblk, NQ), np.int64)
    col = 0
    chunks = []
    for g0 in range(0, nblk, GB):
        g1 = min(g0 + GB, nblk)
        ccol0 = col
        qcols = []
        for q in range(NQ):
            q0 = col
            for b in range(g0, g1):
                off2[b, q] = col
                col += kbq[b, q]
            qcols.append((q0, col))
        chunks.append((g0, g1, ccol0, col, qcols))
    kt = int(col)

    idxg = np.zeros((n_cores, P, kt), np.int32)           # quarter-local rows
    dstl = np.full((n_cores, P, kt), 200.0, np.float32)   # 200 => pad
    # walk the (core, chunk, quarter, block) segments in sorted order
    ptr = 0
    for c in range(n_cores):
        for (g0, g1, _c0, _c1, _qc) in chunks:
            for q in range(NQ):
                for b in range(g0, g1):
                    e = int(cntr[c, b, q])
                    if e == 0:
                        continue
                    j = np.arange(e)
                    pp = j % P
                    kk = j // P + off2[b, q]
                    idxg[c, pp, kk] = so[ptr:ptr + e]
                    dstl[c, pp, kk] = do[ptr:ptr + e]
                    ptr += e
    assert ptr == len(so)

    # int16 index stream: slot i (= col*128 + p) -> (partition i%16, col
    # i//16), replicated over the 8 gpsimd cores
    nslots = kt * P
    idx16 = np.zeros((n_cores, P, nslots // 16), np.int16)
    for c in range(n_cores):
        flat = idxg[c].T.reshape(-1)                     # slot-major
        w = flat.reshape(-1, 16).T.astype(np.int16)      # [16, nslots/16]
        idx16[c] = np.tile(w, (8, 1))

    # x pre-scaled by deg: the resident transposed buffer holds hd = deg*h
    xpad = np.zeros((n_cores, npcp, HID), np.float32)
    degt = np.ones((n_cores, 1, npcp), np.float32)
    scl = np.ones((n_cores, P, nblk), np.float32)
    for c in range(n_cores):
        nid = c * npc + np.arange(npc)
        xpad[c, :npc] = x[c * npc:(c + 1) * npc] * degc[nid][:, None]
        degt[c, 0, :npc] = degc[nid]
        pp = np.arange(npc) % P
        bb = np.arange(npc) // P
        scl[c, pp, bb] = inv[nid]

    # weights, SBUF-layout
    nl = Wl.shape[0]
    wl_h = np.zeros((HID, nl * HID), np.float32)   # [f, l*64+f'] = Wl[l,f,f']
    wr_h = np.zeros((HID, nl * HID), np.float32)
    bl_h = np.zeros((1, nl * HID), np.float32)     # single partition row
    for l in range(nl):
        wl_h[:, l * HID:(l + 1) * HID] = Wl[l]
        wr_h[:, l * HID:(l + 1) * HID] = Wr[l]
        bl_h[0, l * HID:(l + 1) * HID] = bl[l]
    wfct_h = Wfc.reshape(5, HID).T.astype(np.float32)    # [64, 5]
    iota_h = np.broadcast_to(np.arange(P, dtype=np.float32), (P, P)).copy()
    ident_h = np.eye(P, dtype=np.float32)

    ng = npc // 5                      # graphs per core
    gb = (ng + P - 1) // P             # head groups per core

    in_maps = []
    for c in range(n_cores):
        in_maps.append({
            "x_in": xpad[c].astype(BF16),
            "idx16_in": idx16[c],
            "dstl_in": dstl[c].astype(np.float32),
            "scl_in": scl[c],
            "degt_in": degt[c].astype(BF16),
            "iota_in": iota_h.astype(BF16),
            "ident_in": ident_h.astype(BF16),
            "wl_in": wl_h.astype(BF16),
            "wr_in": wr_h.astype(BF16),
            "bl_in": bl_h.astype(BF16),
            "wfct_in": wfct_h.astype(BF16),
        })
    params = dict(kbq=kbq.tolist(), off2=off2.tolist(), kt=kt,
                  chunks=chunks, nblk=nblk, npcp=npcp, npc=npc, hnp=hnp,
                  qrows=qrows, ng=ng, gb=gb,
                  bfc=float(np.asarray(bfc).reshape(-1)[0]),
                  idxg=idxg, dstl_dbg=dstl)
    return in_maps, params


# ---------------------------------------------------------------- program
def build_program(nc, params, n_cores, reps=1, variant='full'):
    import concourse.bass as bass
    import concourse.tile as tile
    from concourse import mybir
    from concourse.tile_rust import add_dep_helper
    from contextlib import ExitStack

    f32 = mybir.dt.float32
    bf16 = mybir.dt.bfloat16
    i16 = mybir.dt.int16
    AF = mybir.ActivationFunctionType
    OP = mybir.AluOpType

    kbq, off2, kt = params["kbq"], params["off2"], params["kt"]
    chunks = params["chunks"]
    nblk, npcp, hnp = params["nblk"], params["npcp"], params["hnp"]
    qrows = params["qrows"]
    ng, gb = params["ng"], params["gb"]
    bfc = params["bfc"]
    nl = N_LAYERS
    nfull = n_cores * npcp
    nhalf = n_cores * hnp
    nslots = kt * P
    hblk = hnp // P                    # dst blocks per half

    x_in = nc.dram_tensor("x_in", [npcp, HID], bf16, kind="ExternalInput")
    idx16_in = nc.dram_tensor("idx16_in", [P, nslots // 16], i16,
                              kind="ExternalInput")
    dstl_in = nc.dram_tensor("dstl_in", [P, kt], f32, kind="ExternalInput")
    scl_in = nc.dram_tensor("scl_in", [P, nblk], f32, kind="ExternalInput")
    degt_in = nc.dram_tensor("degt_in", [1, npcp], bf16, kind="ExternalInput")
    iota_in = nc.dram_tensor("iota_in", [P, P], bf16, kind="ExternalInput")
    ident_in = nc.dram_tensor("ident_in", [P, P], bf16, kind="ExternalInput")
    wl_in = nc.dram_tensor("wl_in", [HID, nl * HID], bf16, kind="ExternalInput")
    wr_in = nc.dram_tensor("wr_in", [HID, nl * HID], bf16, kind="ExternalInput")
    bl_in = nc.dram_tensor("bl_in", [1, nl * HID], bf16, kind="ExternalInput")
    wfct_in = nc.dram_tensor("wfct_in", [HID, 5], bf16, kind="ExternalInput")
    out_t = nc.dram_tensor("out", [P, gb], f32, kind="ExternalOutput")

    groups = [list(range(n_cores))]
    gw = max(c3 - c2 for (_, _, c2, c3, _) in chunks)   # max cols per chunk

    with tile.TileContext(nc) as tc:
        _frees = []
        with ExitStack() as ctx:
            # ---- persistent SBUF
            idx16_sb, _f = tc.tile([P, nslots // 16], i16, name="idx16_sb"); _frees.append(_f)
            dstl_sb, _f = tc.tile([P, kt], f32, name="dstl_sb"); _frees.append(_f)
            scl_sb, _f = tc.tile([P, nblk], f32, name="scl_sb"); _frees.append(_f)
            degt_sb, _f = tc.tile([1, npcp], bf16, name="degt_sb"); _frees.append(_f)
            iota_sb, _f = tc.tile([P, P], bf16, name="iota_sb"); _frees.append(_f)
            ident_sb, _f = tc.tile([P, P], bf16, name="ident_sb"); _frees.append(_f)
            wl_sb, _f = tc.tile([HID, nl * HID], bf16, name="wl_sb"); _frees.append(_f)
            wr_sb, _f = tc.tile([HID, nl * HID], bf16, name="wr_sb"); _frees.append(_f)
            bl_sb, _f = tc.tile([1, nl * HID], bf16, name="bl_sb"); _frees.append(_f)
            wfct_sb, _f = tc.tile([HID, 5], bf16, name="wfct_sb"); _frees.append(_f)
            ht0_sb, _f = tc.tile([HID, nblk * P], bf16, name="ht0_sb"); _frees.append(_f)
            ht1_sb, _f = tc.tile([HID, nblk * P], bf16, name="ht1_sb"); _frees.append(_f)
            hts = [ht0_sb, ht1_sb]

            nc.sync.dma_start(idx16_sb[:], idx16_in[:])
            nc.sync.dma_start(dstl_sb[:], dstl_in[:])
            nc.sync.dma_start(scl_sb[:], scl_in[:])
            nc.sync.dma_start(degt_sb[:], degt_in[:])
            nc.sync.dma_start(iota_sb[:], iota_in[:])
            nc.sync.dma_start(ident_sb[:], ident_in[:])
            nc.sync.dma_start(wl_sb[:], wl_in[:])
            nc.sync.dma_start(wr_sb[:], wr_in[:])
            nc.sync.dma_start(bl_sb[:], bl_in[:])
            nc.sync.dma_start(wfct_sb[:], wfct_in[:])

            # ---- pools
            dram = ctx.enter_context(tc.tile_pool(name="dram", bufs=2, space="DRAM"))
            sb = ctx.enter_context(tc.tile_pool(name="sb", bufs=3))
            gp = ctx.enter_context(tc.tile_pool(name="gp", bufs=3))
            ps = ctx.enter_context(tc.tile_pool(name="ps", bufs=2, space="PSUM"))

            def transpose_to(hb_ap, dst_slice):
                t_ps = ps.tile([HID, P], bf16, tag="tps", name="t_ps")
                nc.tensor.transpose(t_ps[:], hb_ap, ident_sb[:])
                nc.vector.tensor_copy(dst_slice, t_ps[:])

            def produce_y(ht_slice, lw, b, yg_slice):
                y_ps = ps.tile([P, HID], f32, tag="yps", name="y_ps")
                nc.tensor.matmul(y_ps[:], lhsT=ht_slice,
                                 rhs=wl_sb[:, lw * HID:(lw + 1) * HID],
                                 start=True, stop=True)
                nc.scalar.activation(yg_slice, y_ps[:], AF.Copy,
                                     scale=scl_sb[:, b:b + 1])

            def flush_y_half(y_own, half, yh):
                r0 = half * hnp
                dst = y_own[r0:r0 + hnp, :].rearrange("(b p) f -> p b f", p=P)
                nc.sync.dma_start(dst, yh[:])

            split_ag = variant != 'full_ns'
            cc_open = []        # collectives the next gather wave waits on
            qrr = [0]           # SWDGE queue round-robin

            def all_gather_half(y_own, y_full, half):
                if not split_ag:
                    if half == 0:
                        return
                    yi, yo = y_own[:, :], y_full[:, :]
                elif half == 0:
                    yi, yo = y_own[:hnp, :], y_full[:nhalf, :]
                else:
                    yi, yo = y_own[hnp:, :], y_full[nhalf:, :]
                if variant == 'no_coll':
                    nc.sync.dma_start(yo[:hnp, :], yi[:hnp, :])
                else:
                    cc = nc.gpsimd.collective_compute(
                        "AllGather", OP.bypass, replica_groups=groups,
                        ins=[yi.opt()], outs=[yo.opt()])
                    cc_open.append(cc)

            for _rep in range(reps):
              # ---- bootstrap: load x as hd0^T, produce y0, all-gather
              y_own = dram.tile([npcp, HID2], bf16, tag="yown", name="y_own_b")
              y_full = dram.tile([nfull, HID2], bf16, tag="yfull",
                                 name="y_full_b")
              for half in range(2):
                  b0 = half * hblk
                  xh = sb.tile([P, hblk * HID], bf16, tag="xh", name="xh",
                               bufs=1)
                  nc.sync.dma_start(
                      xh[:],
                      x_in[b0 * P:(b0 + hblk) * P, :].rearrange(
                          "(b p) f -> p b f", p=P))
                  yh = gp.tile([P, hblk * HID2], bf16, tag="yh", name="yh",
                               bufs=2)
                  nc.vector.memset(yh[:], 0.0)
                  for b in range(b0, b0 + hblk):
                      j = b - b0
                      transpose_to(xh[:, j * HID:(j + 1) * HID],
                                   ht0_sb[:, b * P:(b + 1) * P])
                      produce_y(ht0_sb[:, b * P:(b + 1) * P], 0, b,
                                yh[:, j * HID2:j * HID2 + HID])
                  flush_y_half(y_own, half, yh)
                  all_gather_half(y_own, y_full, half)

              # ---- layers
              for l in range(nl):
                  last = l == nl - 1
                  ht_prev = hts[l % 2]
                  ht_cur = hts[(l + 1) % 2]
                  if not last:
                      y_own = dram.tile([npcp, HID2], bf16, tag="yown",
                                        name=f"y_own_{l}")
                      y_full_nxt = dram.tile([nfull, HID2], bf16,
                                             tag="yfull", name=f"y_full_{l}")
                      yh = gp.tile([P, hblk * HID2], bf16, tag="yh",
                                   name="yh0", bufs=2)
                      nc.vector.memset(yh[:], 0.0)
                  cc_wait = list(cc_open)
                  cc_open.clear()
                  for (g0, g1, ccol0, ccol1, qcols) in chunks:
                      g_all = gp.tile([P, gw * HID2], bf16, tag="g",
                                      name="g_all", bufs=2)
                      for q in range(NQ):
                          q0, q1 = qcols[q]
                          for c0 in range(q0, q1, 8):
                              c1 = min(c0 + 8, q1)
                              nsub = c1 - c0
                              o_ap = g_all[:, (c0 - ccol0) * HID2:
                                           (c1 - ccol0) * HID2].rearrange(
                                  "p (s e) -> p s e", e=HID2)
                              gth = nc.gpsimd.dma_gather(
                                  o_ap,
                                  y_full[q * qrows:(q + 1) * qrows, :],
                                  idx16_sb[:, c0 * (P // 16):c1 * (P // 16)],
                                  num_idxs=nsub * P,
                                  num_idxs_reg=nsub * P,
                                  elem_size=HID2,
                                  queue_num=qrr[0] % 4)
                              qrr[0] += 1
                              for cc in cc_wait:
                                  add_dep_helper(gth.ins, cc.ins, sync=True,
                                                 reason="gather waits AG")
                      for b in range(g0, g1):
                          a_ps = ps.tile([P, HID], f32, tag="aps", name="a_ps",
                                         bufs=4)
                          first = True
                          for q in range(NQ):
                              o = off2[b][q]
                              for kk in range(kbq[b][q]):
                                  col = o + kk
                                  s_t = sb.tile([P, P], bf16, tag="s",
                                                name="s_t", bufs=16)
                                  nc.vector.tensor_scalar(
                                      s_t[:], iota_sb[:],
                                      dstl_sb[:, col:col + 1],
                                      None, op0=OP.is_equal)
                                  c0 = (col - ccol0) * HID2
                                  nc.tensor.matmul(a_ps[:], lhsT=s_t[:],
                                                   rhs=g_all[:, c0:c0 + HID],
                                                   start=first, stop=False)
                                  first = False
                          # self term: h @ Wr + bl, accumulated in PSUM
                          nc.tensor.matmul(a_ps[:],
                                           lhsT=ht_prev[:, b * P:(b + 1) * P],
                                           rhs=wr_sb[:, l * HID:(l + 1) * HID],
                                           start=False, stop=False)
                          nc.tensor.matmul(a_ps[:],
                                           lhsT=degt_sb[:, b * P:(b + 1) * P],
                                           rhs=bl_sb[:, l * HID:(l + 1) * HID],
                                           start=False, stop=True)
                          hb = sb.tile([P, HID], bf16, tag="hb", name="hb",
                                       bufs=6)
                          if last:
                              nc.scalar.activation(hb[:], a_ps[:], AF.Relu,
                                                   scale=scl_sb[:, b:b + 1])
                          else:
                              nc.scalar.activation(hb[:], a_ps[:], AF.Relu)
                          transpose_to(hb[:], ht_cur[:, b * P:(b + 1) * P])
                          if not last:
                              j = b % hblk
                              produce_y(ht_cur[:, b * P:(b + 1) * P], l + 1,
                                        b, yh[:, j * HID2:j * HID2 + HID])
                              if b == hblk - 1:
                                  flush_y_half(y_own, 0, yh)
                                  all_gather_half(y_own, y_full_nxt, 0)
                                  yh = gp.tile([P, hblk * HID2], bf16,
                                               tag="yh", name="yh1", bufs=2)
                                  nc.vector.memset(yh[:], 0.0)
                              elif b == nblk - 1:
                                  flush_y_half(y_own, 1, yh)
                                  all_gather_half(y_own, y_full_nxt, 1)
                  if not last:
                      y_full = y_full_nxt

              # ---- head: out[g] = sigmoid(sum_j h5[5g+j] . wfc_j + bfc)
              h5t = hts[nl % 2]
              hd_ps = ps.tile([P, gb], f32, tag="aps", name="hd_ps", bufs=4)
              for t in range(gb):
                  gcnt = min(P, ng - t * P)
                  for j in range(5):
                      c0 = 5 * t * P + j
                      lhsT = h5t[:, c0:c0 + 5 * gcnt - 4:5]
                      nc.tensor.matmul(hd_ps[:gcnt, t:t + 1], lhsT=lhsT,
                                       rhs=wfct_sb[:, j:j + 1],
                                       start=(j == 0), stop=(j == 4))
              out_sb = sb.tile([P, gb], f32, tag="outsb", name="out_sb")
              bfc_sb = sb.tile([P, 1], f32, tag="bfc", name="bfc_sb")
              nc.vector.memset(bfc_sb[:], bfc)
              nc.scalar.activation(out_sb[:], hd_ps[:], AF.Sigmoid,
                                   bias=bfc_sb[:])
              nc.sync.dma_start(out_t[:], out_sb[:])

        for _f in reversed(_frees):
            _f()

    return out_t


def make_nc(params, n_cores, enable_asserts=False, reps=1, variant='full'):
    import concourse.bacc as bacc
    nc = bacc.Bacc("TRN2", target_bir_lowering=False, debug=False,
                   enable_asserts=enable_asserts, num_devices=n_cores,
                   num_swdge_queues=4)
    build_program(nc, params, n_cores, reps=reps, variant=variant)
    nc.compile()
    return nc


def assemble_output(results, params, n_cores):
    """results: list (per core) of dicts with 'out' [P, gb] f32."""
    ng, gb = params["ng"], params["gb"]
    out = np.zeros((n_cores * ng, 1), np.float32)
    for c in range(n_cores):
        o = np.asarray(results[c]["out"])          # [P, gb]
        flat = o.T.reshape(-1)[:ng]                # graph g = t*P + p
        out[c * ng:(c + 1) * ng, 0] = flat
    return out


# ---------------------------------------------------------------- entry
def kernel(x, edge_index, Wl, bl, Wr, Wfc, bfc):
    from concourse.bass_utils import run_bass_kernel_spmd

    x = np.asarray(x, dtype=np.float32)
    edge_index = np.asarray(edge_index, dtype=np.int32)
    in_maps, params = prep_host(x, edge_index, np.asarray(Wl), np.asarray(bl),
                                np.asarray(Wr), np.asarray(Wfc),
                                np.asarray(bfc), x.shape[0], N_CORES)
    nc = make_nc(params, N_CORES)
    res = run_bass_kernel_spmd(nc, in_maps, core_ids=list(range(N_CORES)))
    return assemble_output(res.results, params, N_CORES)
